# revision 1
# baseline (speedup 1.0000x reference)
"""Grouped-Query Attention (B=2, T=2048, E=2048, 16 Q heads / 4 KV heads, RoPE,
causal) as a Bass/Tile kernel on 8 Trainium2 NeuronCores.

Sharding: core c = 4*b + h handles batch b (of 2) and KV-head group h (of 4,
i.e. 4 q-heads + 1 kv head).  Each core computes its QKV projections (channel
sharded), RoPE, causal attention for its 4 q heads, and a partial
out-projection over its 512 channels of Wo.  The 4 partial out-proj results per
batch are summed on the host during unsharding (row-sharded out_proj).

Everything on device is channel-major ([channel, token] layout) so matmul
contractions run along the partition axis with wide (512) moving operands.
"""

import os
import sys

import numpy as np

try:
    import concourse.bass as bass
except ModuleNotFoundError:  # fresh grading dir: point at the in-container repo
    for p in ("/opt/trn_rl_repo", "/root/.axon_site/_ro/trn_rl_repo"):
        if os.path.isdir(p) and p not in sys.path:
            sys.path.insert(0, p)
    import concourse.bass as bass

from contextlib import ExitStack

import concourse.tile as tile
from concourse import bacc, mybir
from concourse.bass_utils import run_bass_kernel_spmd

# ---- problem constants (hardcoded per contract) ----
B, T, E = 2, 2048, 2048
N_QHEAD, N_KVHEAD = 16, 4
GROUP = N_QHEAD // N_KVHEAD          # 4 q heads per kv head
D = E // N_QHEAD                     # 128 head dim
KV = E // GROUP                      # 512 kv dim
ROPE_BASE = 10000.0
N_CORES = 8

P = 128                              # partitions
ET = E // P                          # 16 e-tiles
TT = T // P                          # 16 token tiles
TC = 512                             # moving-dim chunk (max for fp32)
NTC = T // TC                        # 4 token chunks

F32 = mybir.dt.float32
USE_F32R = True                      # full-rate fp32r matmuls


def _mm(ap):
    return ap.bitcast(mybir.dt.float32r) if USE_F32R else ap


_CACHE = {}


def _build_program():
    """Build + compile the (SPMD-identical) Bass program once per process."""
    if "nc" in _CACHE:
        return _CACHE["nc"]

    nc = bacc.Bacc("TRN2", target_bir_lowering=False, debug=False,
                   num_devices=N_CORES)

    F32R = mybir.dt.float32r if USE_F32R else F32
    dram = {}
    def din(name, shape, dt=F32):
        dram[name] = nc.dram_tensor(name, list(shape), dt,
                                    kind="ExternalInput").ap()
    din("xt", (ET, P, T), F32R)           # x[b].T tiled: (e, p, t)
    din("wq", (P, ET * GROUP * P), F32R)  # WqT slice, partition-major tiles (e,c)
    din("wk", (P, ET * P), F32R)
    din("wv", (P, ET * P), F32R)
    din("wo", (P, GROUP * TT * P), F32R)  # WoS.T tiles (c,j), partition-major
    din("bias6", (P, 6))            # per-ctile biases: 4x bq, bk, bv
    din("bo16", (P, TT))            # bo per j-tile (zeros on h!=0 cores)
    din("cosq", (P, T))             # rope tables, channel-major, q scaled 1/sqrt(D)
    din("sinq", (P, T))
    din("cosk", (P, T))
    din("sink", (P, T))
    din("ptm", (P, P), F32R)              # rope rotation matrix Pm^T (lhsT)
    din("ident", (P, P))            # identity (for PE transpose)
    din("mask4", (P, GROUP * TC), F32R)   # causal masks, 4 diagonal-straddle tiles
    din("ones1", (P, 8), F32R)
    din("onesr", (1, P), F32R)
    outt = nc.dram_tensor("outt", [TT, P, T], F32, kind="ExternalOutput").ap()

    with tile.TileContext(nc) as tc:
        with ExitStack() as ctx, nc.allow_low_precision(
                reason="fp32r matmul operands; accumulation stays fp32 in PSUM"):
            persist = ctx.enter_context(tc.tile_pool(name="persist", bufs=1))

            def ptile(shape, name, dt=F32):
                return persist.tile(shape, dt, tag=name, name=name)

            # ---------- persistent SBUF tiles ----------
            wq_sb = ptile([P, ET * GROUP * P], "wq_sb", F32R)
            wk_sb = ptile([P, ET * P], "wk_sb", F32R)
            wv_sb = ptile([P, ET * P], "wv_sb", F32R)
            bias6_sb = ptile([P, 8], "bias6_sb")  # padded to 32B
            bo16_sb = ptile([P, TT], "bo16_sb")
            ptm_sb = ptile([P, P], "ptm_sb", F32R)
            ident_sb = ptile([P, P], "ident_sb")
            qT_sb = ptile([P, GROUP * T], "qT_sb")    # 4 heads, channel-major
            kT_sb = ptile([P, T], "kT_sb")
            vT_sb = ptile([P, T], "vT_sb")
            vtok_sb = ptile([P, T], "vtok_sb")        # token-major v
            y_sb = ptile([P, GROUP * T], "y_sb")      # yT per head
            ones1_sb = ptile([P, 8], "ones1_sb", F32R)
            onesr_sb = ptile([1, P], "onesr_sb", F32R)

            # pools (SBUF)
            xw = ctx.enter_context(tc.tile_pool(name="xw", bufs=2))    # x -> wo
            ck = ctx.enter_context(tc.tile_pool(name="ck", bufs=2))    # cos/sin k -> exp
            osb = ctx.enter_context(tc.tile_pool(name="osb", bufs=2))  # out staging
            # pools (PSUM): statically 4 + 4 = 8 banks
            pacc = ctx.enter_context(tc.tile_pool(name="pacc", bufs=4, space="PSUM"))
            pbig = ctx.enter_context(tc.tile_pool(name="pbig", bufs=2, space="PSUM"))

            # ---------- load constants ----------
            nc.sync.dma_start(bias6_sb[:, 0:6], dram["bias6"][:])
            for e in range(ET):
                nc.sync.dma_start(wq_sb[:, e * GROUP * P:(e + 1) * GROUP * P],
                                  dram["wq"][:, e * GROUP * P:(e + 1) * GROUP * P])
            for nm, t in [("wk", wk_sb), ("wv", wv_sb),
                          ("bo16", bo16_sb),
                          ("ptm", ptm_sb), ("ident", ident_sb)]:
                nc.sync.dma_start(t[:], dram[nm][:])
            nc.sync.dma_start(ones1_sb[:], dram["ones1"][:])
            nc.sync.dma_start(onesr_sb[:], dram["onesr"][:])
            cosk_sb = ck.tile([P, T], F32, tag="ck", name="cosk_sb")
            sink_sb = ck.tile([P, T], F32, tag="ck", name="sink_sb")
            nc.sync.dma_start(cosk_sb[:], dram["cosk"][:])
            nc.sync.dma_start(sink_sb[:], dram["sink"][:])

            # ---------- phase 1: QKV projections (channel-major) ----------
            # qT[c,t] = sum_e WqT[e,c] * xT[e,t]  (+bias at evacuation)
            XC = 256                      # x token-chunk width
            NXC = T // XC

            def proj_dst(ct):
                if ct < GROUP:
                    return qT_sb[:, ct * T:(ct + 1) * T]
                return (kT_sb if ct == GROUP else vT_sb)[:, :]

            for xc in range(NXC):
                x_sb = xw.tile([P, ET * XC], F32R, tag="xw", name="x_sb")
                x3 = x_sb[:].rearrange("p (e t) -> p e t", e=ET)
                xd = dram["xt"][:, :, xc * XC:(xc + 1) * XC].rearrange(
                    "e p t -> p e t")
                for q4 in range(4):
                    nc.sync.dma_start(x3[:, q4 * 4:(q4 + 1) * 4, :],
                                      xd[:, q4 * 4:(q4 + 1) * 4, :])
                for half in range(2):          # <=3 live PSUM accums at a time
                    for ct3 in range(3):
                        ct = half * 3 + ct3
                        ppr = pacc.tile([P, XC], F32, tag="acc", name="ppr")
                        for e in range(ET):
                            if ct < GROUP:
                                lhs = wq_sb[:, (e * GROUP + ct) * P:
                                            (e * GROUP + ct + 1) * P]
                            elif ct == GROUP:
                                lhs = wk_sb[:, e * P:(e + 1) * P]
                            else:
                                lhs = wv_sb[:, e * P:(e + 1) * P]
                            nc.tensor.matmul(
                                ppr[:], _mm(lhs),
                                _mm(x_sb[:, e * XC:(e + 1) * XC]),
                                start=(e == 0), stop=(e == ET - 1))
                        dst = proj_dst(ct)
                        dslice = dst[:, xc * XC:(xc + 1) * XC]
                        if ct != GROUP + 1:   # q,k feed fp32r matmuls
                            dslice = _mm(dslice)
                        nc.vector.tensor_scalar_add(
                            dslice, ppr[:], bias6_sb[:, ct:ct + 1])

            # rope q tables reuse the x-chunk slots (x is dead after proj)
            cosq_sb = xw.tile([P, T], F32, tag="xw", name="cosq_sb")
            sinq_sb = xw.tile([P, T], F32, tag="xw", name="sinq_sb")
            nc.sync.dma_start(cosq_sb[:], dram["cosq"][:])
            nc.sync.dma_start(sinq_sb[:], dram["sinq"][:])

            # ---------- phase 1b: RoPE (q scale 1/sqrt(D) folded in tables) ----
            def rope(dst_full, cos_sb, sin_sb):
                for c in range(NTC):
                    cs = slice(c * TC, (c + 1) * TC)
                    rot_ps = pacc.tile([P, TC], F32, tag="acc", name="rot_ps")
                    nc.tensor.matmul(rot_ps[:], _mm(ptm_sb[:]),
                                     _mm(dst_full[:, cs]),
                                     start=True, stop=True)
                    tmp = osb.tile([P, TC], F32, tag="ost", name="tmp", bufs=4)
                    nc.vector.tensor_mul(tmp[:], rot_ps[:], sin_sb[:, cs])
                    nc.vector.tensor_mul(_mm(dst_full[:, cs]),
                                         dst_full[:, cs], cos_sb[:, cs])
                    nc.vector.tensor_add(_mm(dst_full[:, cs]),
                                         dst_full[:, cs], tmp[:])

            rope(kT_sb[:, :], cosk_sb, sink_sb)

            # prefetch out-proj weights now: DMA is idle during attention and
            # the xw slots (x chunks, rope q tables) are dead after rope
            wo_a = xw.tile([P, 2 * TT * P], F32R, tag="xw", name="wo_a")
            wo_b = xw.tile([P, 2 * TT * P], F32R, tag="xw", name="wo_b")
            for half, wt in ((0, wo_a), (1, wo_b)):
                for ct in range(2):
                    off = ct * TT * P
                    nc.sync.dma_start(
                        wt[:, off:off + TT * P],
                        dram["wo"][:, (half * 2 + ct) * TT * P:
                                   (half * 2 + ct + 1) * TT * P])

            # mask tile reuses the rope-tmp slot (rope is done with it)
            mask4_sb = ck.tile([P, GROUP * TC], F32R, tag="rtmp",
                               name="mask4_sb", bufs=1)
            nc.sync.dma_start(mask4_sb[:], dram["mask4"][:])

            # ---------- phase 1c: v -> token-major via PE transpose ----------
            for j in range(TT):
                vps = pacc.tile([P, P], F32, tag="acc", name="vps")
                nc.tensor.transpose(vps[:], vT_sb[:, j * P:(j + 1) * P],
                                    ident_sb[:])
                nc.vector.tensor_copy(_mm(vtok_sb[:, j * P:(j + 1) * P]), vps[:])

            # ---------- phase 2: causal attention per (head, tq-chunk) -------
            # transposed scores: sT[tk, tq] = kT_j^T . qT ; softmax over tk via
            # ones-matmul column sums; normalization folded in at the end.
            # rope of head h+1 (DVE-heavy) overlaps attention of head h
            # (PE-heavy) -- emitted just-in-time per head.
            for h in range(GROUP):
                rope(qT_sb[:, h * T:(h + 1) * T], cosq_sb, sinq_sb)
                for qc in range(NTC):
                    jmax = GROUP * qc + GROUP - 1
                    ng2 = 2 * (qc + 1)          # groups of 2 j-tiles
                    yps = pacc.tile([P, TC], F32, tag="acc", name="yps")
                    sps = pacc.tile([1, TC], F32, tag="acc", name="sps")
                    for g in range(ng2):
                        spsum = pbig.tile([P, 2 * TC], F32, tag="big",
                                          name="spsum")
                        for sub in range(2):
                            j = 2 * g + sub
                            nc.tensor.matmul(
                                spsum[:, sub * TC:(sub + 1) * TC],
                                _mm(kT_sb[:, j * P:(j + 1) * P]),
                                _mm(qT_sb[:, h * T + qc * TC:
                                          h * T + (qc + 1) * TC]),
                                start=True, stop=True)
                        eg = ck.tile([P, 2 * TC], F32R, tag="ck", name="eg")
                        nc.scalar.activation(eg[:], spsum[:],
                                             mybir.ActivationFunctionType.Exp)
                        if g >= ng2 - 2:        # diagonal-straddling groups
                            half = g - (ng2 - 2)
                            nc.vector.tensor_mul(
                                eg[:], eg[:],
                                mask4_sb[:, half * 2 * TC:(half + 1) * 2 * TC])
                        for sub in range(2):
                            j = 2 * g + sub
                            nc.tensor.matmul(
                                yps[:], _mm(vtok_sb[:, j * P:(j + 1) * P]),
                                _mm(eg[:, sub * TC:(sub + 1) * TC]),
                                start=(j == 0), stop=(j == jmax))
                            nc.tensor.matmul(
                                sps[:], _mm(ones1_sb[:, 0:1]),
                                _mm(eg[:, sub * TC:(sub + 1) * TC]),
                                start=(j == 0), stop=(j == jmax))
                    # normalize: y /= colsum (broadcast 1/sum via K=1 matmul)
                    rec = osb.tile([1, TC], F32R, tag="rec", name="rec", bufs=1)
                    nc.vector.reciprocal(rec[:], sps[:])
                    bps = pacc.tile([P, TC], F32, tag="acc", name="bps")
                    nc.tensor.matmul(bps[:], _mm(onesr_sb[:]), _mm(rec[:]),
                                     start=True, stop=True)
                    bcs = osb.tile([P, TC], F32, tag="bc", name="bcs", bufs=1)
                    nc.scalar.copy(bcs[:], bps[:])
                    nc.vector.tensor_mul(
                        _mm(y_sb[:, h * T + qc * TC: h * T + (qc + 1) * TC]),
                        yps[:], bcs[:])

            # ---------- phase 3: partial out-projection ----------
            # outT[j,t] = sum_c WoST[c,j] * yT[c,t]   (+bo on core h==0)
            for jt in range(TT):
                for c in range(NTC):
                    ops = pacc.tile([P, TC], F32, tag="acc", name="ops")
                    for ct in range(GROUP):
                        wo_sb = wo_a if ct < 2 else wo_b
                        cti = ct % 2
                        lhs = wo_sb[:, (cti * TT + jt) * P:
                                    (cti * TT + jt + 1) * P]
                        nc.tensor.matmul(
                            ops[:], _mm(lhs),
                            _mm(y_sb[:, ct * T + c * TC: ct * T + (c + 1) * TC]),
                            start=(ct == 0), stop=(ct == GROUP - 1))
                    ost = osb.tile([P, TC], F32, tag="ost", name="ost", bufs=4)
                    nc.vector.tensor_scalar_add(ost[:], ops[:],
                                                bo16_sb[:, jt:jt + 1])
                    nc.sync.dma_start(outt[jt][:, c * TC:(c + 1) * TC], ost[:])

    nc.compile()
    _CACHE["nc"] = nc
    return nc


def _host_inputs(x, Wq, bq, Wk, bk, Wv, bv, Wo, bo):
    """Per-core input dicts (all fp32, layouts matching the DRAM decls)."""
    f = np.float32
    i = np.arange(1, D // 2 + 1, dtype=np.float64)
    thetas = ROPE_BASE ** (-2.0 * (i - 1.0) / D)
    ang = np.arange(1, T + 1, dtype=np.float64)[:, None] * thetas      # [T, D/2]
    cos = np.concatenate([np.cos(ang), np.cos(ang)], axis=1).T.astype(f)
    sin = np.concatenate([np.sin(ang), np.sin(ang)], axis=1).T.astype(f)
    s = f(1.0 / np.sqrt(D))
    cosq, sinq = np.ascontiguousarray(cos * s), np.ascontiguousarray(sin * s)
    cosk, sink = np.ascontiguousarray(cos), np.ascontiguousarray(sin)

    Pm = np.zeros((D, D), f)
    for d in range(D // 2):
        Pm[d, d + D // 2] = -1.0
        Pm[d + D // 2, d] = 1.0
    ptm = np.ascontiguousarray(Pm.T)
    ident = np.eye(P, dtype=f)

    pcol = np.arange(P)[:, None]
    fcol = np.arange(TC)[None, :]
    mask4 = np.concatenate(
        [(pcol <= fcol - P * r).astype(f) for r in range(GROUP)], axis=1)
    mask4 = np.ascontiguousarray(mask4)

    per_core = []
    for c in range(N_CORES):
        b, h = divmod(c, GROUP)
        xt = np.ascontiguousarray(x[b].T.reshape(ET, P, T))
        WqS = Wq[h * KV:(h + 1) * KV, :]                                # [512, E]
        wq = np.ascontiguousarray(
            WqS.T.reshape(ET, P, GROUP, P).transpose(1, 0, 2, 3).reshape(P, -1))
        WkS = Wk[h * D:(h + 1) * D, :]
        wk = np.ascontiguousarray(
            WkS.T.reshape(ET, P, P).transpose(1, 0, 2).reshape(P, -1))
        WvS = Wv[h * D:(h + 1) * D, :]
        wv = np.ascontiguousarray(
            WvS.T.reshape(ET, P, P).transpose(1, 0, 2).reshape(P, -1))
        WoS = Wo[:, h * KV:(h + 1) * KV]                                # [E, 512]
        wo = np.ascontiguousarray(
            WoS.T.reshape(GROUP, P, TT, P).transpose(1, 0, 2, 3).reshape(P, -1))
        bias6 = np.stack([bq[h * KV + ct * P: h * KV + (ct + 1) * P]
                          for ct in range(GROUP)]
                         + [bk[h * D:(h + 1) * D], bv[h * D:(h + 1) * D]],
                         axis=1).astype(f)
        bo16 = (bo.reshape(TT, P).T if h == 0
                else np.zeros((P, TT), f)).astype(f)
        per_core.append({
            "xt": xt, "wq": wq, "wk": wk, "wv": wv, "wo": wo,
            "bias6": np.ascontiguousarray(bias6),
            "bo16": np.ascontiguousarray(bo16),
            "cosq": cosq, "sinq": sinq, "cosk": cosk, "sink": sink,
            "ptm": ptm, "ident": ident, "mask4": mask4,
            "ones1": np.ones((P, 8), f), "onesr": np.ones((1, P), f),
        })
    return per_core


def kernel(**inputs):
    x = np.asarray(inputs["x"], np.float32)
    nc = _build_program()
    in_maps = _host_inputs(
        x, *(np.asarray(inputs[k], np.float32)
             for k in ("Wq", "bq", "Wk", "bk", "Wv", "bv", "Wo", "bo")))
    res = run_bass_kernel_spmd(nc, in_maps, list(range(N_CORES)))
    out = np.empty((B, T, E), np.float32)
    for b in range(B):
        acc = np.zeros((E, T), np.float32)
        for h in range(GROUP):
            acc += res.results[b * GROUP + h]["outt"].reshape(E, T)
        out[b] = acc.T
    return out

